# revision 34
# baseline (speedup 1.0000x reference)
"""Trainium2 Bass kernel for: out = (x @ wsums.sum(0)) * (1.5 * 0.5).

x: [1024, 8192] f32, wsums: [32, 8192] f32 -> out: [1024, 1] f32.

Sharding across 8 NeuronCores: 8-way along the contraction dim k
(8192 -> 1024 per core).  Each core reads a 4MB x column-shard plus its
128KB wsums k-slice, computes partial dot products for ALL 1024 rows over
its k-slice, and the host sums the 8 per-core partials (the unshard step
for a contraction-sharded dim).  This reads wsums exactly once across the
chip (vs. 8x if replicated) and keeps every per-core DMA descriptor a
4KB-contiguous row slice.

Per-core device program:
  1. DMA wsums slice [32, 1024] -> SBUF.
  2. One PE matmul pair with a constant [32, 128] stationary filled with
     SCALE: reduces the 32 group rows, applies the output scale, AND
     broadcasts the result across all 128 partitions in one shot, directly
     into PSUM -> wp[128, 1024] = SCALE * w_total (no PSUM->SBUF copy).
  3. x streamed as chunks of row-blocks ([128, 1024] each); all chunk
     tiles are SBUF-resident so every DMA is triggered up-front and the
     HBM stream runs at line rate.  Per chunk: one DVE tensor_tensor
     multiply y = x * wp (wp read straight from PSUM; for 2-block chunks
     wp is repeated via a stride-0 broadcast AP instead of being
     materialized twice), then per row-block one ScalarE activation
     (Copy) with accum_out -> the per-partition dot products.
  4. DMA the [128, 8] accumulator block to DRAM.

Environment workarounds (this container's walrus build):
  - it encodes at most ONE semaphore wait per instruction ("Too many sync
    wait commands"), so compile_bir_kernel is wrapped with a BIR post-pass
    that moves excess waits onto preceding same-engine NoOp instructions;
  - it cannot encode bass_isa raw-ISA ops (tensor_tensor_reduce,
    partition_all_reduce, ... -> "ISA wrong length"), so only classic
    mybir ops are used (TensorTensor / Activation / Matmult / Memset).
"""

import json

import numpy as np

import concourse.bass as bass
import concourse.bass2jax as bass2jax
import concourse.bass_utils as bass_utils
import concourse.mybir as mybir
from concourse.tile import TileContext

SCALE = 1.5 * 0.5
B, K, G = 1024, 8192, 32
N_CORES = 8
KSHARD = 8                  # cores along k
BSHARD = N_CORES // KSHARD  # cores along batch
KB = K // KSHARD            # per-core k width
BB = B // BSHARD            # per-core rows
P = 128
NBLK = BB // P              # row-blocks per core
F32 = mybir.dt.float32

# Set by test.py to profile; results stashed in LAST_RESULTS.
TRACE = False
TRACE_KWARGS = {}
LAST_RESULTS = None

_built = None

# ---------------------------------------------------------------------------
# Workaround: this container's walrus encodes at most 1 sync wait per
# instruction.  Split longer on_wait lists onto preceding same-engine NoOps.
MAX_WAITS = 1
_orig_compile_bir_kernel = bass_utils.compile_bir_kernel


def _split_waits_in_bir(bir: dict) -> int:
    counter = [0]

    def fix_blocks(blocks):
        for bb in blocks:
            out = []
            for ins in bb.get("instructions", []):
                si = ins.get("sync_info")
                ow = (si or {}).get("on_wait") or []
                if len(ow) > MAX_WAITS:
                    extra, keep = ow[:-MAX_WAITS], ow[-MAX_WAITS:]
                    for i in range(0, len(extra), MAX_WAITS):
                        counter[0] += 1
                        out.append({
                            "name": f"I-waitsplit-{counter[0]}",
                            "engine": ins["engine"],
                            "opcode": "NoOp",
                            "ins": [],
                            "outs": [],
                            "debug": ins.get("debug", 0),
                            "sync_info": {
                                "on_update": [],
                                "on_wait": extra[i : i + MAX_WAITS],
                            },
                        })
                    si["on_wait"] = keep
                out.append(ins)
            bb["instructions"] = out
            if bb.get("blocks"):
                fix_blocks(bb["blocks"])

    for fn in bir["functions"]:
        fix_blocks(fn["blocks"])
    return counter[0]


def _patched_compile_bir_kernel(bir_json, tmpdir, neff_name="file.neff"):
    if isinstance(bir_json, str):
        bir_json = bir_json.encode()
    bir = json.loads(bir_json)
    _split_waits_in_bir(bir)
    return _orig_compile_bir_kernel(json.dumps(bir).encode(), tmpdir, neff_name)


bass_utils.compile_bir_kernel = _patched_compile_bir_kernel
bass2jax.compile_bir_kernel = _patched_compile_bir_kernel


# ---------------------------------------------------------------------------
# Overlapped TileContext exit.  The stock exit serializes: drain(+DMA-sem
# waits) -> all-engine barrier -> sem clears -> barrier, so every engine's
# ~3-6us walrus postamble (each engine zeroes a fixed 51-semaphore slice:
# Tensor S[3-53], Scalar S[54-104], GpSimd S[105-155], Vector S[156-206],
# Sync S[207-255]) starts only after the out-DMA's ~2us completion receipt.
# This kernel's live semaphores (Tile range ~151-174: barrier, engine
# clocks, DMAHW lanes) fall ONLY in the GpSimd and Vector slices, so:
#   - Tensor and Scalar get no tail instructions at all -> their postambles
#     run as soon as their body ends (PE finishes ~14us in!);
#   - Sync drains with the global-clock + DMA-completion waits, then incs a
#     handoff semaphore;
#   - GpSimd and Vector wait for the handoff before entering their
#     postambles (so the DMAHW/clock sems they zero are no longer in use).
# The explicit Tile sem clears are dropped: the walrus postamble wipes all
# 256 semaphores every execution, which keeps re-execution correct.
import concourse.tile as tile_mod
from concourse.tile import TileContext as _TC


def _overlap_drain_and_barrier(self, tick_clock, wait_clock):
    nc = self.nc
    drain_inst = nc.sync.drain()
    wait_clock.add_sem_waits(
        drain_inst.ins,
        tile_mod.ScopedClock({None: tick_clock.global_clock}),
    )
    done = nc.alloc_semaphore("tail_dma_done")
    # Must not sit in Tensor's or Scalar's postamble-clear slice (they are
    # released early and would zero it while GpSimd/Vector still wait).
    assert done.num >= 105, done.num
    drain_inst.then_inc(done, 1)
    nc.gpsimd.wait_ge(done, 1)
    nc.vector.wait_ge(done, 1)
    popped = nc._tile_sem_poison_stack.pop()
    assert popped is self._sem_poison


_TC._drain_and_barrier = _overlap_drain_and_barrier
# ---------------------------------------------------------------------------


def _build():
    nc = bass.Bass("TRN2")
    x_sh = nc.dram_tensor("x_shard", (BB, KB), F32, kind="ExternalInput")
    w_sh = nc.dram_tensor("wsums_shard", (G, KB), F32, kind="ExternalInput")
    out = nc.dram_tensor("out_acc", (P, NBLK), F32, kind="ExternalOutput")

    with TileContext(nc) as tc:
        with (
            tc.tile_pool(name="const", bufs=1) as cpool,
            tc.tile_pool(name="xbuf", bufs=max(3, NBLK)) as xpool,
            tc.tile_pool(name="ybuf", bufs=2) as ypool,
            tc.tile_pool(name="psum", bufs=1, space="PSUM") as ppool,
        ):
            ws = cpool.tile([G, KB], F32)
            nc.sync.dma_start(out=ws, in_=w_sh.ap())

            # Stationary = SCALE (not 1.0): folds the output scale into the
            # broadcast matmul, so wp = SCALE * w_total.
            ones = cpool.tile([G, P], F32)
            nc.gpsimd.memset(ones, SCALE)

            # wp[m, n] = sum_g ones[g, m] * ws[g, n] = SCALE*w_total[n] on
            # every partition m.  N<=512 per matmul (one PSUM bank each).
            wp = ppool.tile([P, KB], F32)
            for j in range(KB // 512):
                nc.tensor.matmul(
                    wp[:, j * 512 : (j + 1) * 512],
                    ones,
                    ws[:, j * 512 : (j + 1) * 512],
                    start=True,
                    stop=True,
                )

            acc = cpool.tile([P, NBLK], F32)
            # All chunk tiles resident so every DMA is triggered up-front
            # and the HBM stream runs at line rate.  Chunks are processed in
            # arrival order, with single-block tail chunks; the block-0
            # chunk is moved to the END of the stream so the final chunk's
            # ~2us DMA-completion receipt hides under the TT/ACT work of
            # the chunk that arrived just before it.
            if NBLK == 8:
                chunks = [(1, 2), (3, 4), (5, 6), (7,), (0,)]
            elif NBLK % 2 == 0:
                chunks = [tuple(range(2 * j, 2 * j + 2)) for j in range(NBLK // 2)]
            else:
                chunks = [(j,) for j in range(NBLK)]
            assert sorted(b for c in chunks for b in c) == list(range(NBLK))

            for blocks in chunks:
                nrb = len(blocks)
                rb0 = blocks[0]
                assert blocks == tuple(range(rb0, rb0 + nrb))
                xt = xpool.tile([P, nrb * KB], F32, tag=f"xt{nrb}")
                # src[p, a, k] = x_shard[(rb0 + a) * P + p, k]
                src = bass.AP(
                    x_sh,
                    rb0 * P * KB,
                    [[KB, P], [P * KB, nrb], [1, KB]],
                )
                nc.sync.dma_start(out=xt, in_=src)
                yt = ypool.tile([P, nrb * KB], F32, tag=f"yt{nrb}")
                if nrb == 1:
                    nc.vector.tensor_tensor(yt, xt, wp, op=mybir.AluOpType.mult)
                else:
                    # One fused multiply over nrb row-blocks; wp is repeated
                    # along a stride-0 middle dim instead of being
                    # materialized nrb times.
                    x3 = xt[:].rearrange("p (a k) -> p a k", a=nrb)
                    y3 = yt[:].rearrange("p (a k) -> p a k", a=nrb)
                    wb = wp[:].unsqueeze(1).broadcast_to([P, nrb, KB])
                    nc.vector.tensor_tensor(y3, x3, wb, op=mybir.AluOpType.mult)
                for a in range(nrb):
                    if blocks is chunks[-1] and a == nrb - 1:
                        # Final block: reduce on DVE so the trailing ScalarE
                        # accumulate chain and the last reduce run on
                        # different engines and finish together.
                        nc.vector.tensor_reduce(
                            acc[:, rb0 + a : rb0 + a + 1],
                            yt[:, a * KB : (a + 1) * KB],
                            axis=mybir.AxisListType.X,
                            op=mybir.AluOpType.add,
                        )
                    else:
                        nc.scalar.activation(
                            yt[:, a * KB : (a + 1) * KB],
                            yt[:, a * KB : (a + 1) * KB],
                            mybir.ActivationFunctionType.Copy,
                            accum_out=acc[:, rb0 + a : rb0 + a + 1],
                        )

            nc.sync.dma_start(out=out.ap(), in_=acc)
    return nc


def kernel(x: np.ndarray, wsums: np.ndarray) -> np.ndarray:
    global _built, LAST_RESULTS
    if _built is None:
        _built = _build()
    nc = _built

    x = np.ascontiguousarray(np.asarray(x, dtype=np.float32))
    wsums = np.ascontiguousarray(np.asarray(wsums, dtype=np.float32))

    in_maps = []
    for c in range(N_CORES):
        bb_i, kb_i = divmod(c, KSHARD)
        xs = np.ascontiguousarray(
            x[bb_i * BB : (bb_i + 1) * BB, kb_i * KB : (kb_i + 1) * KB]
        )
        wsl = np.ascontiguousarray(wsums[:, kb_i * KB : (kb_i + 1) * KB])
        in_maps.append({"x_shard": xs, "wsums_shard": wsl})

    res = bass_utils.run_bass_kernel_spmd(
        nc,
        in_maps,
        core_ids=list(range(N_CORES)),
        trace=TRACE,
        **TRACE_KWARGS,
    )
    LAST_RESULTS = res

    parts = []
    for bb_i in range(BSHARD):
        tot = None
        for kb_i in range(KSHARD):
            acc = res.results[bb_i * KSHARD + kb_i]["out_acc"]  # [P, NBLK]
            vec = acc.T.reshape(BB)  # row 128*j + p  <-  acc[p, j]
            tot = vec if tot is None else tot + vec
        parts.append(tot)
    return np.concatenate(parts).astype(np.float32)[:, None]


# revision 35
# speedup vs baseline: 1.0325x; 1.0325x over previous
"""Trainium2 Bass kernel for: out = (x @ wsums.sum(0)) * (1.5 * 0.5).

x: [1024, 8192] f32, wsums: [32, 8192] f32 -> out: [1024, 1] f32.

Sharding across 8 NeuronCores: 8-way along the contraction dim k
(8192 -> 1024 per core).  Each core reads a 4MB x column-shard plus its
128KB wsums k-slice, computes partial dot products for ALL 1024 rows over
its k-slice, and the host sums the 8 per-core partials (the unshard step
for a contraction-sharded dim).  This reads wsums exactly once across the
chip (vs. 8x if replicated) and keeps every per-core DMA descriptor a
4KB-contiguous row slice.

Per-core device program:
  1. DMA wsums slice [32, 1024] -> SBUF.
  2. One PE matmul pair with a constant [32, 128] stationary filled with
     SCALE: reduces the 32 group rows, applies the output scale, AND
     broadcasts the result across all 128 partitions in one shot, directly
     into PSUM -> wp[128, 1024] = SCALE * w_total (no PSUM->SBUF copy).
  3. x streamed as chunks of row-blocks ([128, 1024] each); all chunk
     tiles are SBUF-resident so every DMA is triggered up-front and the
     HBM stream runs at line rate.  Per chunk: one DVE tensor_tensor
     multiply y = x * wp (wp read straight from PSUM; for 2-block chunks
     wp is repeated via a stride-0 broadcast AP instead of being
     materialized twice), then per row-block one ScalarE activation
     (Copy) with accum_out -> the per-partition dot products.
  4. DMA the [128, 8] accumulator block to DRAM.

Environment workarounds (this container's walrus build):
  - it encodes at most ONE semaphore wait per instruction ("Too many sync
    wait commands"), so compile_bir_kernel is wrapped with a BIR post-pass
    that moves excess waits onto preceding same-engine NoOp instructions;
  - it cannot encode bass_isa raw-ISA ops (tensor_tensor_reduce,
    partition_all_reduce, ... -> "ISA wrong length"), so only classic
    mybir ops are used (TensorTensor / Activation / Matmult / Memset).
"""

import json

import numpy as np

import concourse.bass as bass
import concourse.bass2jax as bass2jax
import concourse.bass_utils as bass_utils
import concourse.mybir as mybir
from concourse.tile import TileContext

SCALE = 1.5 * 0.5
B, K, G = 1024, 8192, 32
N_CORES = 8
KSHARD = 8                  # cores along k
BSHARD = N_CORES // KSHARD  # cores along batch
KB = K // KSHARD            # per-core k width
BB = B // BSHARD            # per-core rows
P = 128
NBLK = BB // P              # row-blocks per core
F32 = mybir.dt.float32

# Set by test.py to profile; results stashed in LAST_RESULTS.
TRACE = False
TRACE_KWARGS = {}
LAST_RESULTS = None

_built = None

# ---------------------------------------------------------------------------
# Workaround: this container's walrus encodes at most 1 sync wait per
# instruction.  Split longer on_wait lists onto preceding same-engine NoOps.
MAX_WAITS = 1
_orig_compile_bir_kernel = bass_utils.compile_bir_kernel


def _split_waits_in_bir(bir: dict) -> int:
    counter = [0]

    def fix_blocks(blocks):
        for bb in blocks:
            out = []
            for ins in bb.get("instructions", []):
                si = ins.get("sync_info")
                ow = (si or {}).get("on_wait") or []
                if len(ow) > MAX_WAITS:
                    extra, keep = ow[:-MAX_WAITS], ow[-MAX_WAITS:]
                    for i in range(0, len(extra), MAX_WAITS):
                        counter[0] += 1
                        out.append({
                            "name": f"I-waitsplit-{counter[0]}",
                            "engine": ins["engine"],
                            "opcode": "NoOp",
                            "ins": [],
                            "outs": [],
                            "debug": ins.get("debug", 0),
                            "sync_info": {
                                "on_update": [],
                                "on_wait": extra[i : i + MAX_WAITS],
                            },
                        })
                    si["on_wait"] = keep
                out.append(ins)
            bb["instructions"] = out
            if bb.get("blocks"):
                fix_blocks(bb["blocks"])

    for fn in bir["functions"]:
        fix_blocks(fn["blocks"])
    return counter[0]


def _patched_compile_bir_kernel(bir_json, tmpdir, neff_name="file.neff"):
    if isinstance(bir_json, str):
        bir_json = bir_json.encode()
    bir = json.loads(bir_json)
    _split_waits_in_bir(bir)
    return _orig_compile_bir_kernel(json.dumps(bir).encode(), tmpdir, neff_name)


bass_utils.compile_bir_kernel = _patched_compile_bir_kernel
bass2jax.compile_bir_kernel = _patched_compile_bir_kernel


# ---------------------------------------------------------------------------
# Overlapped TileContext exit.  The stock exit serializes: drain(+DMA-sem
# waits) -> all-engine barrier -> sem clears -> barrier, so every engine's
# ~3-6us walrus postamble (each engine zeroes a fixed 51-semaphore slice:
# Tensor S[3-53], Scalar S[54-104], GpSimd S[105-155], Vector S[156-206],
# Sync S[207-255]) starts only after the out-DMA's ~2us completion receipt.
# This kernel's live semaphores (Tile range ~151-174: barrier, engine
# clocks, DMAHW lanes) fall ONLY in the GpSimd and Vector slices, so:
#   - Tensor and Scalar get no tail instructions at all -> their postambles
#     run as soon as their body ends (PE finishes ~14us in!);
#   - Sync drains with the global-clock + DMA-completion waits, then incs a
#     handoff semaphore;
#   - GpSimd and Vector wait for the handoff before entering their
#     postambles (so the DMAHW/clock sems they zero are no longer in use).
# The explicit Tile sem clears are dropped: the walrus postamble wipes all
# 256 semaphores every execution, which keeps re-execution correct.
import concourse.tile as tile_mod
from concourse.tile import TileContext as _TC


def _overlap_drain_and_barrier(self, tick_clock, wait_clock):
    nc = self.nc
    drain_inst = nc.sync.drain()
    wait_clock.add_sem_waits(
        drain_inst.ins,
        tile_mod.ScopedClock({None: tick_clock.global_clock}),
    )
    done = nc.alloc_semaphore("tail_dma_done")
    # Must not sit in Tensor's or Scalar's postamble-clear slice (they are
    # released early and would zero it while GpSimd/Vector still wait).
    assert done.num >= 105, done.num
    drain_inst.then_inc(done, 1)
    nc.gpsimd.wait_ge(done, 1)
    nc.vector.wait_ge(done, 1)
    popped = nc._tile_sem_poison_stack.pop()
    assert popped is self._sem_poison


_TC._drain_and_barrier = _overlap_drain_and_barrier
# ---------------------------------------------------------------------------


def _build():
    # Bass.__init__ ends with an all-engine barrier ordering its const-AP
    # memsets (fp32 0/1, bf16 1, u8 127) against the body.  This kernel
    # never reads those const APs, and the NRT start barrier already aligns
    # the engines at execution start, so skip it: Sync reaches the first
    # x-DMA trigger ~1.7us earlier.
    _orig_aeb = bass.Bass.all_engine_barrier
    bass.Bass.all_engine_barrier = lambda self, **kw: None
    try:
        nc = bass.Bass("TRN2")
    finally:
        bass.Bass.all_engine_barrier = _orig_aeb
    x_sh = nc.dram_tensor("x_shard", (BB, KB), F32, kind="ExternalInput")
    w_sh = nc.dram_tensor("wsums_shard", (G, KB), F32, kind="ExternalInput")
    out = nc.dram_tensor("out_acc", (P, NBLK), F32, kind="ExternalOutput")

    with TileContext(nc) as tc:
        with (
            tc.tile_pool(name="const", bufs=1) as cpool,
            tc.tile_pool(name="xbuf", bufs=max(3, NBLK)) as xpool,
            tc.tile_pool(name="ybuf", bufs=2) as ypool,
            tc.tile_pool(name="psum", bufs=1, space="PSUM") as ppool,
        ):
            ws = cpool.tile([G, KB], F32)
            nc.sync.dma_start(out=ws, in_=w_sh.ap())

            # Stationary = SCALE (not 1.0): folds the output scale into the
            # broadcast matmul, so wp = SCALE * w_total.
            ones = cpool.tile([G, P], F32)
            nc.gpsimd.memset(ones, SCALE)

            # wp[m, n] = sum_g ones[g, m] * ws[g, n] = SCALE*w_total[n] on
            # every partition m.  N<=512 per matmul (one PSUM bank each).
            wp = ppool.tile([P, KB], F32)
            for j in range(KB // 512):
                nc.tensor.matmul(
                    wp[:, j * 512 : (j + 1) * 512],
                    ones,
                    ws[:, j * 512 : (j + 1) * 512],
                    start=True,
                    stop=True,
                )

            acc = cpool.tile([P, NBLK], F32)
            # All chunk tiles resident so every DMA is triggered up-front
            # and the HBM stream runs at line rate.  Chunks are processed in
            # arrival order, with single-block tail chunks; the block-0
            # chunk is moved to the END of the stream so the final chunk's
            # ~2us DMA-completion receipt hides under the TT/ACT work of
            # the chunk that arrived just before it.
            if NBLK == 8:
                chunks = [(1, 2), (3, 4), (5, 6), (7,), (0,)]
            elif NBLK % 2 == 0:
                chunks = [tuple(range(2 * j, 2 * j + 2)) for j in range(NBLK // 2)]
            else:
                chunks = [(j,) for j in range(NBLK)]
            assert sorted(b for c in chunks for b in c) == list(range(NBLK))

            for blocks in chunks:
                nrb = len(blocks)
                rb0 = blocks[0]
                assert blocks == tuple(range(rb0, rb0 + nrb))
                xt = xpool.tile([P, nrb * KB], F32, tag=f"xt{nrb}")
                # src[p, a, k] = x_shard[(rb0 + a) * P + p, k]
                src = bass.AP(
                    x_sh,
                    rb0 * P * KB,
                    [[KB, P], [P * KB, nrb], [1, KB]],
                )
                nc.sync.dma_start(out=xt, in_=src)
                yt = ypool.tile([P, nrb * KB], F32, tag=f"yt{nrb}")
                if nrb == 1:
                    nc.vector.tensor_tensor(yt, xt, wp, op=mybir.AluOpType.mult)
                else:
                    # One fused multiply over nrb row-blocks; wp is repeated
                    # along a stride-0 middle dim instead of being
                    # materialized nrb times.
                    x3 = xt[:].rearrange("p (a k) -> p a k", a=nrb)
                    y3 = yt[:].rearrange("p (a k) -> p a k", a=nrb)
                    wb = wp[:].unsqueeze(1).broadcast_to([P, nrb, KB])
                    nc.vector.tensor_tensor(y3, x3, wb, op=mybir.AluOpType.mult)
                for a in range(nrb):
                    if blocks is chunks[-1] and a == nrb - 1:
                        # Final block: reduce on DVE so the trailing ScalarE
                        # accumulate chain and the last reduce run on
                        # different engines and finish together.
                        nc.vector.tensor_reduce(
                            acc[:, rb0 + a : rb0 + a + 1],
                            yt[:, a * KB : (a + 1) * KB],
                            axis=mybir.AxisListType.X,
                            op=mybir.AluOpType.add,
                        )
                    else:
                        nc.scalar.activation(
                            yt[:, a * KB : (a + 1) * KB],
                            yt[:, a * KB : (a + 1) * KB],
                            mybir.ActivationFunctionType.Copy,
                            accum_out=acc[:, rb0 + a : rb0 + a + 1],
                        )

            nc.sync.dma_start(out=out.ap(), in_=acc)
    return nc


def kernel(x: np.ndarray, wsums: np.ndarray) -> np.ndarray:
    global _built, LAST_RESULTS
    if _built is None:
        _built = _build()
    nc = _built

    x = np.ascontiguousarray(np.asarray(x, dtype=np.float32))
    wsums = np.ascontiguousarray(np.asarray(wsums, dtype=np.float32))

    in_maps = []
    for c in range(N_CORES):
        bb_i, kb_i = divmod(c, KSHARD)
        xs = np.ascontiguousarray(
            x[bb_i * BB : (bb_i + 1) * BB, kb_i * KB : (kb_i + 1) * KB]
        )
        wsl = np.ascontiguousarray(wsums[:, kb_i * KB : (kb_i + 1) * KB])
        in_maps.append({"x_shard": xs, "wsums_shard": wsl})

    res = bass_utils.run_bass_kernel_spmd(
        nc,
        in_maps,
        core_ids=list(range(N_CORES)),
        trace=TRACE,
        **TRACE_KWARGS,
    )
    LAST_RESULTS = res

    parts = []
    for bb_i in range(BSHARD):
        tot = None
        for kb_i in range(KSHARD):
            acc = res.results[bb_i * KSHARD + kb_i]["out_acc"]  # [P, NBLK]
            vec = acc.T.reshape(BB)  # row 128*j + p  <-  acc[p, j]
            tot = vec if tot is None else tot + vec
        parts.append(tot)
    return np.concatenate(parts).astype(np.float32)[:, None]
